# revision 13
# baseline (speedup 1.0000x reference)
"""Min-max normalization kernel (nn_EstimationSTD) for 8 Trainium2 cores.

Reference computation (x: (16,1,3,1024,1024) f32):
    f0   = x[:,:,0] flattened to (16384, 1024)          # frame 0
    f2   = x[:,:,2] flattened to (16384, 1024)          # frame 2
    sout = where(row < 1024, f2 - f0, f0)               # diff only in batch 0
    mn/mx = per-column min/max over all 16384 rows
    out  = (sout - mn) / where(mx-mn == 0, 1, mx-mn)    # (16,1,1024,1024)

Strategy: shard COLUMNS across the 8 cores (128 columns each). The host
transposes so each core gets a contiguous [128 cols, 16384 rows] block with
columns on SBUF partitions; the per-column min/max becomes a free-axis
reduction that is fully core-local (no collectives needed).

The min/max pair is fused into ONE single-pass custom DVE op:
    body      = select(Idx < N-1, x, running_max(x))
    out       = x stream whose LAST element is replaced by the global max
    accum_out = min(body) = min over x[0..N-2]
so one 1x-rate pass yields both stats; two tiny fix-up ops recover the
last raw element for the min and gather the per-chunk maxes.
"""

import sys

import numpy as np

_REPO = "/opt/trn_rl_repo"
if _REPO not in sys.path:
    sys.path.insert(0, _REPO)

import concourse.bacc as bacc
import concourse.mybir as mybir
import concourse.tile as tile
from concourse.bass_utils import run_bass_kernel_spmd

N_CORES = 8
BS, C, NF, H, W = 16, 1, 3, 1024, 1024
R = BS * C * H          # 16384 rows (bs*c*h)
PC = W // N_CORES       # 128 columns per core -> SBUF partitions
CH = 2048               # rows per chunk along the free axis
NCH = R // CH           # 8 chunks
F32 = mybir.dt.float32
ALU = mybir.AluOpType

OP_NAME = "MINMAX_SCAN_ANT"


def _minmax_ref(in0, in1, c0, c1, c2):
    sc = np.maximum.accumulate(np.asarray(in0, np.float32), axis=-1)
    idx = np.arange(in0.shape[-1])
    out = np.where(idx < c0, in0, sc)
    acc = np.minimum(out.min(axis=-1), np.float32(c1))
    return out, acc


def _register_minmax_op():
    import concourse.dve_ops as dve_ops
    from concourse.dve_spec import (
        Spec, Src0, C0, C1, Idx, AluOp, scan, select, minn, lower,
    )
    from concourse.dve_uop import DveOpSpec

    if OP_NAME in dve_ops._SUB_OPCODE_FOR_NAME:
        return getattr(dve_ops, OP_NAME)

    spec = Spec(
        body=select(Idx < C0, Src0, scan(AluOp.MAX, Src0)),
        accum=minn,
        accum_init=C1,
        reference=_minmax_ref,
    )
    row = dve_ops._CUSTOM_DVE_ROW_BASE + len(dve_ops.OPS)
    assert row < 0x20
    shas = {}
    for ver in ("v3", "v4"):
        s = DveOpSpec(name=OP_NAME, opcode=row, uops=lower(spec, ver=ver), rd1_en=False)
        shas[ver] = s.sha(ver)
    op = dve_ops.DveOp(OP_NAME, spec, subdim=False, uops_sha=shas)
    dve_ops.OPS.append(op)
    dve_ops.CUSTOM_DVE_SPECS[OP_NAME] = spec
    dve_ops._SUB_OPCODE_FOR_NAME[OP_NAME] = row
    setattr(dve_ops, OP_NAME, op)
    return op


_NC_CACHE = {}


def _build_nc():
    minmax_op = _register_minmax_op()

    nc = bacc.Bacc(
        "TRN2",
        target_bir_lowering=False,
        debug=False,
        num_devices=N_CORES,
    )
    a = nc.dram_tensor("a_t", [PC, R], F32, kind="ExternalInput")   # frame0^T slice
    b = nc.dram_tensor("b_t", [PC, H], F32, kind="ExternalInput")   # frame2[batch0]^T slice
    o = nc.dram_tensor("o_t", [PC, R], F32, kind="ExternalOutput")

    with tile.TileContext(nc) as tc:
        with (
            tc.tile_pool(name="big", bufs=1) as big_pool,
            tc.tile_pool(name="small", bufs=1) as small_pool,
        ):
            A = big_pool.tile([PC, R], F32, tag="A")       # data, resident
            S = big_pool.tile([PC, R], F32, tag="S")       # scan sink
            bt = small_pool.tile([PC, H], F32, tag="bt")
            mins = small_pool.tile([PC, 24], F32, tag="mins")
            gmin = small_pool.tile([PC, 1], F32, tag="gmin")
            gmax = small_pool.tile([PC, 1], F32, tag="gmax")
            rng = small_pool.tile([PC, 1], F32, tag="rng")
            eq = small_pool.tile([PC, 1], F32, tag="eq")
            denom = small_pool.tile([PC, 1], F32, tag="denom")
            inv = small_pool.tile([PC, 1], F32, tag="inv")

            # split every transfer across BOTH HWDGE rings (sync + scalar):
            # HBM reads are latency-bound, two rings double the outstanding
            # request depth
            HC = CH // 2
            nc.sync.dma_start(out=bt[:, 0 : H // 2], in_=b[:, 0 : H // 2])
            nc.scalar.dma_start(out=bt[:, H // 2 : H], in_=b[:, H // 2 : H])
            for i in range(NCH):
                lo = i * CH
                nc.sync.dma_start(out=A[:, lo : lo + HC], in_=a[:, lo : lo + HC])
                nc.scalar.dma_start(
                    out=A[:, lo + HC : lo + CH], in_=a[:, lo + HC : lo + CH]
                )

            # rows [0, H) are batch 0: sout = f2 - f0 (in place)
            nc.vector.tensor_sub(A[:, 0:H], bt[:, :], A[:, 0:H])

            # fused single-pass min+max per chunk:
            #   S[:, sl] = data except last element := chunk max
            #   mins[:, i] = min over chunk's first CH-1 elements
            for i in range(NCH):
                sl = slice(i * CH, (i + 1) * CH)
                nc.vector._custom_dve(
                    minmax_op,
                    out=S[:, sl],
                    in0=A[:, sl],
                    s0=float(CH - 1),
                    s1=3.4e38,
                    accum_out=mins[:, i : i + 1],
                )

            # fix-ups: the chunks' last raw elements for the min; the
            # per-chunk maxes sit at S[:, (i+1)*CH-1]
            nc.vector.tensor_scalar(
                out=mins[:, 16:24], in0=A[:, CH - 1 :: CH], scalar1=0.0, scalar2=None,
                op0=ALU.bypass, op1=ALU.min, accum_out=mins[:, 8:9],
            )
            nc.vector.tensor_scalar(
                out=mins[:, 0:9], in0=mins[:, 0:9], scalar1=0.0, scalar2=None,
                op0=ALU.bypass, op1=ALU.min, accum_out=gmin[:, 0:1],
            )
            nc.vector.tensor_scalar(
                out=mins[:, 16:24], in0=S[:, CH - 1 :: CH], scalar1=0.0, scalar2=None,
                op0=ALU.bypass, op1=ALU.max, accum_out=gmax[:, 0:1],
            )

            nc.vector.tensor_sub(rng[:, :], gmax[:, 0:1], gmin[:, 0:1])
            # sklearn _handle_zeros_in_scale: range 0 -> divide by 1
            nc.vector.tensor_scalar(
                out=eq[:, :], in0=rng[:, :], scalar1=0.0, scalar2=None,
                op0=ALU.is_equal,
            )
            nc.vector.tensor_add(denom[:, :], rng[:, :], eq[:, :])
            nc.vector.reciprocal(inv[:, :], denom[:, :])

            # normalize: out = (sout - gmin) * inv, then store (both rings)
            for i in range(NCH):
                sl = slice(i * CH, (i + 1) * CH)
                lo = i * CH
                nc.vector.tensor_scalar(
                    out=A[:, sl], in0=A[:, sl],
                    scalar1=gmin[:, 0:1], scalar2=inv[:, 0:1],
                    op0=ALU.subtract, op1=ALU.mult,
                )
                nc.sync.dma_start(out=o[:, lo : lo + HC], in_=A[:, lo : lo + HC])
                nc.scalar.dma_start(
                    out=o[:, lo + HC : lo + CH], in_=A[:, lo + HC : lo + CH]
                )

    nc.compile()
    return nc


def get_nc():
    if "nc" not in _NC_CACHE:
        _NC_CACHE["nc"] = _build_nc()
    return _NC_CACHE["nc"]


def _make_in_maps(x):
    x = np.asarray(x, dtype=np.float32)
    assert x.shape == (BS, C, NF, H, W), x.shape
    f0 = x[:, 0, 0, :, :].reshape(BS * H, W)       # (16384, 1024) frame 0
    f2b0 = x[0, 0, 2, :, :]                        # (1024, 1024) frame 2, batch 0
    f0T = np.ascontiguousarray(f0.T)               # (1024, 16384)
    f2T = np.ascontiguousarray(f2b0.T)             # (1024, 1024) [w, h]
    in_maps = []
    for i in range(N_CORES):
        ws = slice(PC * i, PC * (i + 1))
        in_maps.append({
            "a_t": np.ascontiguousarray(f0T[ws]),
            "b_t": np.ascontiguousarray(f2T[ws]),
        })
    return in_maps


def _assemble(results):
    outT = np.concatenate([results[i]["o_t"] for i in range(N_CORES)], axis=0)
    return np.ascontiguousarray(outT.T).reshape(BS, C, H, W).astype(np.float32, copy=False)


def run(x, **spmd_kwargs):
    """Run on hardware; returns (output, BassKernelResults)."""
    nc = get_nc()
    res = run_bass_kernel_spmd(
        nc, _make_in_maps(x), core_ids=list(range(N_CORES)), **spmd_kwargs
    )
    return _assemble(res.results), res


def kernel(x):
    out, _ = run(x)
    return out


# revision 17
# speedup vs baseline: 1.0196x; 1.0196x over previous
"""Min-max normalization kernel (nn_EstimationSTD) for 8 Trainium2 cores.

Reference computation (x: (16,1,3,1024,1024) f32):
    f0   = x[:,:,0] flattened to (16384, 1024)          # frame 0
    f2   = x[:,:,2] flattened to (16384, 1024)          # frame 2
    sout = where(row < 1024, f2 - f0, f0)               # diff only in batch 0
    mn/mx = per-column min/max over all 16384 rows
    out  = (sout - mn) / where(mx-mn == 0, 1, mx-mn)    # (16,1,1024,1024)

Strategy: shard COLUMNS across the 8 cores (128 columns each). The host
transposes so each core gets a contiguous [128 cols, 16384 rows] block with
columns on SBUF partitions; the per-column min/max becomes a free-axis
reduction that is fully core-local (no collectives needed).

The min/max pair is fused into ONE single-pass custom DVE op:
    body      = select(Idx < N-1, x, running_max(x))
    out       = x stream whose LAST element is replaced by the global max
    accum_out = min(body) = min over x[0..N-2]
so one 1x-rate pass yields both stats; two tiny fix-up ops recover the
last raw element for the min and gather the per-chunk maxes.
"""

import sys

import numpy as np

_REPO = "/opt/trn_rl_repo"
if _REPO not in sys.path:
    sys.path.insert(0, _REPO)

import concourse.bacc as bacc
import concourse.mybir as mybir
import concourse.tile as tile
from concourse.bass_utils import run_bass_kernel_spmd

N_CORES = 8
BS, C, NF, H, W = 16, 1, 3, 1024, 1024
R = BS * C * H          # 16384 rows (bs*c*h)
PC = W // N_CORES       # 128 columns per core -> SBUF partitions
CH = 2048               # rows per chunk along the free axis
NCH = R // CH           # 8 chunks
F32 = mybir.dt.float32
ALU = mybir.AluOpType

OP_NAME = "MINMAX_SCAN_ANT"


def _minmax_ref(in0, in1, c0, c1, c2):
    sc = np.maximum.accumulate(np.asarray(in0, np.float32), axis=-1)
    idx = np.arange(in0.shape[-1])
    out = np.where(idx < c0, in0, sc)
    acc = np.minimum(out.min(axis=-1), np.float32(c1))
    return out, acc


def _register_minmax_op():
    import concourse.dve_ops as dve_ops
    from concourse.dve_spec import (
        Spec, Src0, C0, C1, Idx, AluOp, scan, select, minn, lower,
    )
    from concourse.dve_uop import DveOpSpec

    if OP_NAME in dve_ops._SUB_OPCODE_FOR_NAME:
        return getattr(dve_ops, OP_NAME)

    spec = Spec(
        body=select(Idx < C0, Src0, scan(AluOp.MAX, Src0)),
        accum=minn,
        accum_init=C1,
        reference=_minmax_ref,
    )
    row = dve_ops._CUSTOM_DVE_ROW_BASE + len(dve_ops.OPS)
    assert row < 0x20
    shas = {}
    for ver in ("v3", "v4"):
        s = DveOpSpec(name=OP_NAME, opcode=row, uops=lower(spec, ver=ver), rd1_en=False)
        shas[ver] = s.sha(ver)
    op = dve_ops.DveOp(OP_NAME, spec, subdim=False, uops_sha=shas)
    dve_ops.OPS.append(op)
    dve_ops.CUSTOM_DVE_SPECS[OP_NAME] = spec
    dve_ops._SUB_OPCODE_FOR_NAME[OP_NAME] = row
    setattr(dve_ops, OP_NAME, op)
    return op


_NC_CACHE = {}


def _build_nc():
    minmax_op = _register_minmax_op()

    nc = bacc.Bacc(
        "TRN2",
        target_bir_lowering=False,
        debug=False,
        num_devices=N_CORES,
    )
    # chunk-major DRAM layout: each [PC, CH] chunk is one contiguous 1MB
    # block, so every DMA is a fully sequential HBM stream
    a = nc.dram_tensor("a_t", [NCH, PC, CH], F32, kind="ExternalInput")
    b = nc.dram_tensor("b_t", [PC, H], F32, kind="ExternalInput")
    o = nc.dram_tensor("o_t", [NCH, PC, CH], F32, kind="ExternalOutput")

    with tile.TileContext(nc) as tc:
        with (
            tc.tile_pool(name="big", bufs=1) as big_pool,
            tc.tile_pool(name="small", bufs=1) as small_pool,
        ):
            A = big_pool.tile([PC, R], F32, tag="A")       # data, resident
            S = big_pool.tile([PC, R], F32, tag="S")       # scan sink
            bt = small_pool.tile([PC, H], F32, tag="bt")
            mins = small_pool.tile([PC, 24], F32, tag="mins")
            gmin = small_pool.tile([PC, 1], F32, tag="gmin")
            gmax = small_pool.tile([PC, 1], F32, tag="gmax")
            rng = small_pool.tile([PC, 1], F32, tag="rng")
            eq = small_pool.tile([PC, 1], F32, tag="eq")
            denom = small_pool.tile([PC, 1], F32, tag="denom")
            inv = small_pool.tile([PC, 1], F32, tag="inv")

            nc.sync.dma_start(out=bt[:, :], in_=b[:, :])
            for i in range(NCH):
                sl = slice(i * CH, (i + 1) * CH)
                nc.sync.dma_start(out=A[:, sl], in_=a[i, :, :])

            # rows [0, H) are batch 0: sout = f2 - f0 (in place)
            nc.vector.tensor_sub(A[:, 0:H], bt[:, :], A[:, 0:H])

            # fused single-pass min+max per chunk:
            #   S[:, sl] = data except last element := chunk max
            #   mins[:, i] = min over chunk's first CH-1 elements
            for i in range(NCH):
                sl = slice(i * CH, (i + 1) * CH)
                nc.vector._custom_dve(
                    minmax_op,
                    out=S[:, sl],
                    in0=A[:, sl],
                    s0=float(CH - 1),
                    s1=3.4e38,
                    accum_out=mins[:, i : i + 1],
                )

            # fix-ups: the chunks' last raw elements for the min; the
            # per-chunk maxes sit at S[:, (i+1)*CH-1]
            nc.vector.tensor_scalar(
                out=mins[:, 16:24], in0=A[:, CH - 1 :: CH], scalar1=0.0, scalar2=None,
                op0=ALU.bypass, op1=ALU.min, accum_out=mins[:, 8:9],
            )
            nc.vector.tensor_scalar(
                out=mins[:, 0:9], in0=mins[:, 0:9], scalar1=0.0, scalar2=None,
                op0=ALU.bypass, op1=ALU.min, accum_out=gmin[:, 0:1],
            )
            nc.vector.tensor_scalar(
                out=mins[:, 16:24], in0=S[:, CH - 1 :: CH], scalar1=0.0, scalar2=None,
                op0=ALU.bypass, op1=ALU.max, accum_out=gmax[:, 0:1],
            )

            nc.vector.tensor_sub(rng[:, :], gmax[:, 0:1], gmin[:, 0:1])
            # sklearn _handle_zeros_in_scale: range 0 -> divide by 1
            nc.vector.tensor_scalar(
                out=eq[:, :], in0=rng[:, :], scalar1=0.0, scalar2=None,
                op0=ALU.is_equal,
            )
            nc.vector.tensor_add(denom[:, :], rng[:, :], eq[:, :])
            nc.vector.reciprocal(inv[:, :], denom[:, :])

            # normalize: out = (sout - gmin) * inv, then store. Stores go on
            # the scalar-engine HWDGE ring, separate FIFO from the loads.
            for i in range(NCH):
                sl = slice(i * CH, (i + 1) * CH)
                nc.vector.tensor_scalar(
                    out=A[:, sl], in0=A[:, sl],
                    scalar1=gmin[:, 0:1], scalar2=inv[:, 0:1],
                    op0=ALU.subtract, op1=ALU.mult,
                )
                nc.scalar.dma_start(out=o[i, :, :], in_=A[:, sl])

    nc.compile()
    return nc


def get_nc():
    if "nc" not in _NC_CACHE:
        _NC_CACHE["nc"] = _build_nc()
    return _NC_CACHE["nc"]


def _make_in_maps(x):
    x = np.asarray(x, dtype=np.float32)
    assert x.shape == (BS, C, NF, H, W), x.shape
    f0 = x[:, 0, 0, :, :].reshape(BS * H, W)       # (16384, 1024) frame 0
    f2b0 = x[0, 0, 2, :, :]                        # (1024, 1024) frame 2, batch 0
    f0T = np.ascontiguousarray(f0.T)               # (1024, 16384)
    f2T = np.ascontiguousarray(f2b0.T)             # (1024, 1024) [w, h]
    in_maps = []
    for i in range(N_CORES):
        ws = slice(PC * i, PC * (i + 1))
        # chunk-major: [PC, R] -> [NCH, PC, CH]
        a_cm = np.ascontiguousarray(
            f0T[ws].reshape(PC, NCH, CH).transpose(1, 0, 2)
        )
        in_maps.append({
            "a_t": a_cm,
            "b_t": np.ascontiguousarray(f2T[ws]),
        })
    return in_maps


def _assemble(results):
    # per-core [NCH, PC, CH] -> [PC, R]; stack cores -> [W, R]
    outT = np.concatenate(
        [
            results[i]["o_t"].transpose(1, 0, 2).reshape(PC, R)
            for i in range(N_CORES)
        ],
        axis=0,
    )
    return np.ascontiguousarray(outT.T).reshape(BS, C, H, W).astype(np.float32, copy=False)


def run(x, **spmd_kwargs):
    """Run on hardware; returns (output, BassKernelResults)."""
    nc = get_nc()
    res = run_bass_kernel_spmd(
        nc, _make_in_maps(x), core_ids=list(range(N_CORES)), **spmd_kwargs
    )
    return _assemble(res.results), res


def kernel(x):
    out, _ = run(x)
    return out
